# revision 6
# baseline (speedup 1.0000x reference)
"""2-layer LSTM (B=32, T=256, N=M=1024) on 8 Trainium2 NeuronCores.

Strategy: gate/hidden-dim sharding across the 8 cores (128 hidden units and
512 gate rows per core per layer), weights resident in SBUF as f32r.
  - Phase 1 (on device): P0 = x @ Wih0^T + b0 for all timesteps, sharded by
    gate rows, stored transposed-to-[token, gate] in DRAM.
  - Phase 2: per-step recurrence. Each core computes its gate slice
    gates = P0[t] + h_prev @ Whh^T (h-stationary matmuls, f32r), applies the
    LSTM cell elementwise for its 128 hidden units, transposes its h slice to
    [hid, batch] and AllGathers it so every core has the full h^T for the
    next step's matmul. Layer 1 is pipelined one step behind layer 0.
All matmul operands use float32r (~tf32 precision, 4x fp32 matmul speed);
cell-state arithmetic stays fp32.
"""
import os
import sys

import numpy as np

if "/opt/trn_rl_repo" not in sys.path:
    sys.path.insert(0, "/opt/trn_rl_repo")

L, B, T_FULL, N, M = 2, 32, 256, 1024, 1024
NCORES = 8
HS = M // NCORES          # hidden slice per core: 128
G = 4 * HS                # gate rows per core per layer: 512
KT = M // 128             # k-tiles over the hidden/feature dim: 8

_PROG_CACHE = {}


def _build_program(T):
    import concourse.bacc as bacc
    import concourse.mybir as mybir
    import concourse.tile as tile
    import concourse.masks as masks

    F32 = mybir.dt.float32
    F32R = mybir.dt.float32r
    AF = mybir.ActivationFunctionType
    NTOK = T * B

    nc = bacc.Bacc("TRN2", target_bir_lowering=False, debug=False,
                   num_devices=NCORES)

    # ---- I/O ----
    xT = nc.dram_tensor("xT", [N, NTOK], F32R, kind="ExternalInput")
    w0t = nc.dram_tensor("w0t", [N, G], F32R, kind="ExternalInput")
    r0t = nc.dram_tensor("r0t", [M, G], F32R, kind="ExternalInput")
    w1t = nc.dram_tensor("w1t", [M, G], F32R, kind="ExternalInput")
    r1t = nc.dram_tensor("r1t", [M, G], F32R, kind="ExternalInput")
    b0 = nc.dram_tensor("b0", [1, G], F32R, kind="ExternalInput")
    b1 = nc.dram_tensor("b1", [1, G], F32R, kind="ExternalInput")
    h0T0 = nc.dram_tensor("h0T0", [M, B], F32R, kind="ExternalInput")
    h1T0 = nc.dram_tensor("h1T0", [M, B], F32R, kind="ExternalInput")
    c0s = nc.dram_tensor("c0s", [B, HS], F32, kind="ExternalInput")
    c1s = nc.dram_tensor("c1s", [B, HS], F32, kind="ExternalInput")

    oh = nc.dram_tensor("oh", [T, HS, B], F32, kind="ExternalOutput")
    hf = nc.dram_tensor("hf", [L, HS, B], F32, kind="ExternalOutput")
    cf = nc.dram_tensor("cf", [L, B, HS], F32, kind="ExternalOutput")
    debug = bool(int(os.environ.get("LSTM_DEBUG", "0")))
    if debug:
        dbg_p0 = nc.dram_tensor("dbg_p0", [B, G], F32, kind="ExternalOutput")
        dbg_g0 = nc.dram_tensor("dbg_g0", [B, G], F32, kind="ExternalOutput")
        dbg_h0f = nc.dram_tensor("dbg_h0f", [128, KT * B], F32, kind="ExternalOutput")
        dbg_hT0 = nc.dram_tensor("dbg_hT0", [HS, B], F32, kind="ExternalOutput")

    rg = [list(range(NCORES))]

    with tile.TileContext(nc) as tc:
        from contextlib import ExitStack
        es = ExitStack()
        cpool = es.enter_context(tc.tile_pool(name="const", bufs=1))
        dram = es.enter_context(tc.tile_pool(name="dram", bufs=1, space="DRAM"))

        # ---- constants / weights resident in SBUF ----
        id32f = cpool.tile([32, 32], F32)
        masks.make_identity(nc, id32f[:])
        id32r = cpool.tile([32, 32], F32R)
        nc.vector.tensor_copy(id32r[:], id32f[:])
        id128f = cpool.tile([128, 128], F32)
        masks.make_identity(nc, id128f[:])
        ones_f = cpool.tile([1, G], F32)
        nc.vector.memset(ones_f[:], 1.0)
        ones_gr = cpool.tile([1, G], F32R)
        nc.vector.tensor_copy(ones_gr[:], ones_f[:])
        ones_br = cpool.tile([1, B], F32R)
        nc.vector.tensor_copy(ones_br[:], ones_f[:1, :B])

        sw0 = cpool.tile([128, KT * G], F32R)
        sr0 = cpool.tile([128, KT * G], F32R)
        sw1 = cpool.tile([128, KT * G], F32R)
        sr1 = cpool.tile([128, KT * G], F32R)
        for k in range(KT):
            nc.sync.dma_start(sw0[:, G * k:G * (k + 1)], w0t[128 * k:128 * (k + 1), :])
            nc.sync.dma_start(sr0[:, G * k:G * (k + 1)], r0t[128 * k:128 * (k + 1), :])
            nc.sync.dma_start(sw1[:, G * k:G * (k + 1)], w1t[128 * k:128 * (k + 1), :])
            nc.sync.dma_start(sr1[:, G * k:G * (k + 1)], r1t[128 * k:128 * (k + 1), :])
        sb0 = cpool.tile([1, G], F32R)
        nc.sync.dma_start(sb0[:], b0[:])
        sb1 = cpool.tile([1, G], F32R)
        nc.sync.dma_start(sb1[:], b1[:])

        p0store = dram.tile([NTOK, G], F32R)

        # ---- Phase 1: P0 = x @ Wih0^T + b0, stored as [token, gate] ----
        CH = 512 if NTOK % 512 == 0 else B  # tokens per chunk
        NCH = NTOK // CH
        with tc.tile_pool(name="pre_xt", bufs=3) as xtp, \
             tc.tile_pool(name="pre_ps", bufs=2, space="PSUM") as psp, \
             tc.tile_pool(name="pre_sb", bufs=5) as sbp, \
             tc.tile_pool(name="pre_pst", bufs=2, space="PSUM") as pstp, \
             tc.tile_pool(name="pre_out", bufs=3) as outp:
            for n in range(NCH):
                xts = []
                for k in range(KT):
                    xt_k = xtp.tile([128, CH], F32R, tag=f"xt{k}")
                    nc.sync.dma_start(xt_k[:], xT[128 * k:128 * (k + 1),
                                                  CH * n:CH * (n + 1)])
                    xts.append(xt_k)
                sms = []
                for m in range(4):
                    psA = psp.tile([128, CH], F32, tag="psA")
                    for k in range(KT):
                        nc.tensor.matmul(psA[:],
                                         sw0[:, G * k + 128 * m:G * k + 128 * (m + 1)],
                                         xts[k][:],
                                         start=(k == 0), stop=False)
                    nc.tensor.matmul(psA[:], sb0[0:1, 128 * m:128 * (m + 1)],
                                     ones_gr[0:1, 0:CH], start=False, stop=True)
                    s_m = sbp.tile([128, CH], F32, tag=f"sm{m}")
                    if m % 2 == 0:
                        nc.scalar.copy(s_m[:], psA[:])
                    else:
                        nc.vector.tensor_copy(s_m[:], psA[:])
                    sms.append(s_m)
                for q in range((CH + 127) // 128):
                    qw = min(128, CH - 128 * q)
                    psT = pstp.tile([128, 512], F32, tag="psT")
                    for m in range(4):
                        nc.tensor.transpose(psT[0:qw, 128 * m:128 * (m + 1)],
                                            sms[m][:, 128 * q:128 * q + qw],
                                            id128f[:])
                    oT = outp.tile([128, 512], F32R, tag="oT")
                    if q % 2 == 0:
                        nc.vector.tensor_copy(oT[0:qw, :], psT[0:qw, :])
                    else:
                        nc.scalar.copy(oT[0:qw, :], psT[0:qw, :])
                    nc.sync.dma_start(
                        p0store[CH * n + 128 * q:CH * n + 128 * q + qw, :],
                        oT[0:qw, :])

        # ---- Phase 2: recurrence ----
        es2 = ExitStack()
        hfp = es2.enter_context(tc.tile_pool(name="hfull", bufs=3))
        gps = es2.enter_context(tc.tile_pool(name="gates", bufs=2, space="PSUM"))
        trp = es2.enter_context(tc.tile_pool(name="trps", bufs=2, space="PSUM"))
        ap = es2.enter_context(tc.tile_pool(name="act", bufs=2))
        cp = es2.enter_context(tc.tile_pool(name="cstate", bufs=2))
        htp = es2.enter_context(tc.tile_pool(name="hT", bufs=2))
        p0p = es2.enter_context(tc.tile_pool(name="p0c", bufs=6))
        dcc = es2.enter_context(tc.tile_pool(name="dcc", bufs=2, space="DRAM"))

        def load_hfull(src_dram, tag):
            hfull = hfp.tile([128, KT * B], F32R, tag=tag)
            for k in range(KT):
                nc.sync.dma_start(hfull[:, B * k:B * (k + 1)],
                                  src_dram[128 * k:128 * (k + 1), :])
            return hfull

        h0f = load_hfull(h0T0, "h0f")
        h1f = load_hfull(h1T0, "h1f")
        c0t = cp.tile([B, HS], F32, tag="c0")
        nc.sync.dma_start(c0t[:], c0s[:])
        c1t = cp.tile([B, HS], F32, tag="c1")
        nc.sync.dma_start(c1t[:], c1s[:])

        state = {"h0f": h0f, "h1f": h1f, "c0": c0t, "c1": c1t,
                 "hT0": None, "hT1": None}

        def cell(layer, ps, c_prev):
            """LSTM cell elementwise on the gate slice; returns (c_new, hT_slice)."""
            lt = f"l{layer}"
            s_ifo = ap.tile([B, 3 * HS], F32, tag=f"ifo{lt}")
            nc.scalar.activation(s_ifo[:], ps[:, 0:3 * HS], AF.Sigmoid)
            s_tg = ap.tile([B, HS], F32, tag=f"tg{lt}")
            nc.scalar.activation(s_tg[:], ps[:, 3 * HS:4 * HS], AF.Tanh)
            t1 = ap.tile([B, HS], F32, tag=f"t1{lt}")
            nc.vector.tensor_mul(t1[:], s_ifo[:, 0:HS], s_tg[:])
            t2 = ap.tile([B, HS], F32, tag=f"t2{lt}")
            nc.vector.tensor_mul(t2[:], s_ifo[:, HS:2 * HS], c_prev[:])
            c_new = cp.tile([B, HS], F32, tag=f"c{layer}")
            nc.vector.tensor_add(c_new[:], t1[:], t2[:])
            s_tc = ap.tile([B, HS], F32, tag=f"tc{lt}")
            nc.scalar.activation(s_tc[:], c_new[:], AF.Tanh)
            h_s = ap.tile([B, HS], F32, tag=f"hs{lt}")
            nc.vector.tensor_mul(h_s[:], s_ifo[:, 2 * HS:3 * HS], s_tc[:])
            psT = trp.tile([HS, B], F32, tag=f"tr{lt}")
            nc.tensor.transpose(psT[:], h_s[:], id32f[:])
            hTs = htp.tile([HS, B], F32R, tag=f"hT{lt}")
            nc.vector.tensor_copy(hTs[:], psT[:])
            return c_new, hTs

        def allgather(hTs, layer):
            lt = f"l{layer}"
            cc_in = dcc.tile([HS, B], F32R, tag=f"ci{lt}")
            nc.sync.dma_start(cc_in[:], hTs[:])
            cc_out = dcc.tile([M, B], F32R, tag=f"co{lt}", addr_space="Shared")
            nc.gpsimd.collective_compute(
                "AllGather", mybir.AluOpType.bypass,
                ins=[cc_in.opt()], outs=[cc_out.opt()],
                replica_groups=rg)
            hfull = hfp.tile([128, KT * B], F32R, tag=f"h{layer}f")
            for k in range(KT):
                nc.sync.dma_start(hfull[:, B * k:B * (k + 1)],
                                  cc_out[128 * k:128 * (k + 1), :])
            return hfull

        def l0_step(t):
            p0c = p0p.tile([B, G], F32R, tag="p0c")
            nc.sync.dma_start(p0c[:], p0store[B * t:B * (t + 1), :])
            ps = gps.tile([B, G], F32, tag="g0")
            for k in range(KT):
                nc.tensor.matmul(ps[:], state["h0f"][:, B * k:B * (k + 1)],
                                 sr0[:, G * k:G * (k + 1)],
                                 start=(k == 0), stop=False)
            nc.tensor.matmul(ps[:], id32r[:], p0c[:], start=False, stop=True)
            if debug and t == 0:
                nc.sync.dma_start(dbg_p0[:], p0c[:].bitcast(F32))
                gsb = ap.tile([B, G], F32, tag="dbgg")
                nc.vector.tensor_copy(gsb[:], ps[:])
                nc.sync.dma_start(dbg_g0[:], gsb[:])
                nc.sync.dma_start(dbg_h0f[:], state["h0f"][:].bitcast(F32))
            c_new, hTs = cell(0, ps, state["c0"])
            if debug and t == 0:
                nc.sync.dma_start(dbg_hT0[:], hTs[:].bitcast(F32))
            state["c0"] = c_new
            state["hT0"] = hTs
            state["h0f"] = allgather(hTs, 0)

        def l1_step(t, h0f_t):
            ps = gps.tile([B, G], F32, tag="g1")
            for k in range(KT):
                nc.tensor.matmul(ps[:], h0f_t[:, B * k:B * (k + 1)],
                                 sw1[:, G * k:G * (k + 1)],
                                 start=(k == 0), stop=False)
            for k in range(KT):
                nc.tensor.matmul(ps[:], state["h1f"][:, B * k:B * (k + 1)],
                                 sr1[:, G * k:G * (k + 1)],
                                 start=False, stop=False)
            nc.tensor.matmul(ps[:], ones_br[0:1, :], sb1[0:1, :],
                             start=False, stop=True)
            c_new, hTs = cell(1, ps, state["c1"])
            state["c1"] = c_new
            state["hT1"] = hTs
            nc.sync.dma_start(oh[t], hTs[:].bitcast(F32))
            state["h1f"] = allgather(hTs, 1)

        l0_step(0)
        h0_for_l1 = state["h0f"]
        for t in range(1, T):
            l0_step(t)
            l1_step(t - 1, h0_for_l1)
            h0_for_l1 = state["h0f"]
        l1_step(T - 1, h0_for_l1)

        nc.sync.dma_start(hf[0], state["hT0"][:].bitcast(F32))
        nc.sync.dma_start(hf[1], state["hT1"][:].bitcast(F32))
        nc.sync.dma_start(cf[0], state["c0"][:])
        nc.sync.dma_start(cf[1], state["c1"][:])

        es2.close()
        es.close()

    nc.compile()
    return nc


def _get_program(T):
    if T not in _PROG_CACHE:
        _PROG_CACHE[T] = _build_program(T)
    return _PROG_CACHE[T]


def _prep_inputs(x, h, c, Wih, Whh, bih, bhh):
    """Host-side layout prep: per-core weight slices (transposed), xT, h^T."""
    T = x.shape[1]
    # xT[f, t*B+b] = x[b, t, f]
    xT = np.ascontiguousarray(x.transpose(2, 1, 0).reshape(N, T * B))
    bsum = (bih + bhh).astype(np.float32)
    in_maps = []
    for j in range(NCORES):
        r = np.arange(HS * j, HS * (j + 1))
        perm = np.concatenate([r, M + r, 3 * M + r, 2 * M + r])  # [i|f|o|g]
        m = {
            "xT": xT,
            "w0t": np.ascontiguousarray(Wih[0][perm].T),
            "r0t": np.ascontiguousarray(Whh[0][perm].T),
            "w1t": np.ascontiguousarray(Wih[1][perm].T),
            "r1t": np.ascontiguousarray(Whh[1][perm].T),
            "b0": np.ascontiguousarray(bsum[0][perm][None, :]),
            "b1": np.ascontiguousarray(bsum[1][perm][None, :]),
            "h0T0": np.ascontiguousarray(h[0].T),
            "h1T0": np.ascontiguousarray(h[1].T),
            "c0s": np.ascontiguousarray(c[0][:, r]),
            "c1s": np.ascontiguousarray(c[1][:, r]),
        }
        in_maps.append(m)
    return in_maps


def _assemble(results, T):
    outs = np.empty((B, T, M), np.float32)
    h_final = np.empty((L, B, M), np.float32)
    c_final = np.empty((L, B, M), np.float32)
    for j, r in enumerate(results):
        sl = slice(HS * j, HS * (j + 1))
        outs[:, :, sl] = r["oh"].transpose(2, 0, 1)        # [T,HS,B] -> [B,T,HS]
        h_final[:, :, sl] = r["hf"].transpose(0, 2, 1)     # [L,HS,B] -> [L,B,HS]
        c_final[:, :, sl] = r["cf"]                        # [L,B,HS]
    return outs, h_final, c_final


_LAST_RESULT = {}


def kernel(x, h, c, Wih, Whh, bih, bhh):
    from concourse import bass_utils

    x = np.asarray(x, dtype=np.float32)
    h = np.asarray(h, dtype=np.float32)
    c = np.asarray(c, dtype=np.float32)
    Wih = np.asarray(Wih, dtype=np.float32)
    Whh = np.asarray(Whh, dtype=np.float32)
    bih = np.asarray(bih, dtype=np.float32)
    bhh = np.asarray(bhh, dtype=np.float32)

    T = x.shape[1]
    nc = _get_program(T)
    in_maps = _prep_inputs(x, h, c, Wih, Whh, bih, bhh)
    trace = bool(int(os.environ.get("LSTM_TRACE", "0")))
    kw = {}
    if trace:
        kw = dict(trace=True, tmpdir=os.environ.get("LSTM_TRACE_DIR", None))
    res = bass_utils.run_bass_kernel_spmd(nc, in_maps, list(range(NCORES)), **kw)
    _LAST_RESULT["res"] = res
    return _assemble(res.results, T)
